# revision 12
# baseline (speedup 1.0000x reference)
"""GAT (2-layer, 8-head) Trainium2 kernel, 8-core SPMD.

Phase 1: head-parallel — core h computes head h's GAT layer over the full
  graph.  Uses the identity  exp(lrelu(f1_i + f2_j)) = v1_i * max(w_i*u2_j, v2_j)
  with w = exp((1-a)f1), u2 = exp(f2), v2 = exp(a*f2); the v1_i factor is a
  per-row scale that cancels in the softmax normalization, so the [N,N]
  unnormalized attention needs ONE 4x-mode tensor_scalar (mult+max against
  two per-partition scalars) and ONE 2x-mode tensor_tensor mask multiply per
  128-row tile (mask multiply split between DVE and GpSimd).  The tiny f1/f2
  vectors (x @ (W@a), O(N*F)) are folded on the host like the W@a folds, so
  the attention stream starts immediately.  A zero-weight dummy matmul gates
  each 4-tile block of PE work so the tensor engine runs in long bursts and
  ramps out of its low-power state.
Phase 2: row-parallel — host gathers h_T [512, N] (bf16), every core computes
  the full Wh2 = h@W_out chunk-by-chunk interleaved with its attention
  matmuls (keeps PE continuously busy), then elu + log_softmax (no max-shift;
  logits are small) for its own N/8-row slice.
"""

import sys

for p in ("/opt/trn_rl_repo", "/opt/pypackages"):
    if p not in sys.path:
        sys.path.append(p)

import numpy as np
import ml_dtypes

import concourse.bass as bass
import concourse.bacc as bacc
import concourse.tile as tile
from concourse import mybir
from concourse.bass_utils import run_bass_kernel_spmd
from concourse.masks import make_identity

BF16 = mybir.dt.bfloat16
F32 = mybir.dt.float32
AX = mybir.AxisListType
OP = mybir.AluOpType
AF = mybir.ActivationFunctionType

N, FIN, HID, HEADS, FOUT = 4096, 512, 64, 8, 256
NCORES = 8
ALPHA = 0.2


def _flat_write_ap(t, rows, cols):
    return bass.AP(tensor=t, offset=0, ap=[[cols, rows], [1, cols]])


def build_phase1(n=N, fin=FIN, hid=HID):
    """Per-core: xT [fin, n] bf16, maskT [n, n] bf16, wcat [fin, hid] bf16,
    wvec [n] bf16 (= exp((1-a)f1)), u2i/v2i [128, n/128] f32
    -> hout [hid, n] bf16 (elu'd, normalized head features, transposed)."""
    nc = bacc.Bacc("TRN2", target_bir_lowering=False, debug=False,
                   enable_asserts=False)
    kch = fin // 128          # contraction chunks for x@W
    nch = n // 128            # 128-row chunks of nodes
    nib = max(1, n // 512)    # 512-col i-blocks for PSUM banks
    ibw = min(n, 512)

    xT = nc.dram_tensor("xT", [fin, n], BF16, kind="ExternalInput")
    maskT = nc.dram_tensor("maskT", [n, n], BF16, kind="ExternalInput")
    wcat = nc.dram_tensor("wcat", [fin, hid], BF16, kind="ExternalInput")
    wvec = nc.dram_tensor("wvec", [n], BF16, kind="ExternalInput")
    u2i = nc.dram_tensor("u2i", [128, nch], F32, kind="ExternalInput")
    v2i = nc.dram_tensor("v2i", [128, nch], F32, kind="ExternalInput")
    hout = nc.dram_tensor("hout", [hid, n], BF16, kind="ExternalOutput")
    scrR = nc.dram_tensor("scrR", [n], BF16)
    scrRb = nc.dram_tensor("scrRb", [n], BF16)

    with tile.TileContext(nc) as tc:
        with tc.tile_pool(name="consts", bufs=1) as consts:
            wsb = consts.tile([128, kch, hid], BF16)
            for kc in range(kch):
                nc.sync.dma_start(out=wsb[:, kc, :], in_=wcat[kc * 128:(kc + 1) * 128, :])
            u2c = consts.tile([128, nch], F32)
            v2c = consts.tile([128, nch], F32)
            nc.sync.dma_start(out=u2c[:], in_=u2i[:, :])
            nc.sync.dma_start(out=v2c[:], in_=v2i[:, :])
            w1b = consts.tile([128, n], BF16)
            for q4 in range(4):
                qn = n // 4
                bap = bass.AP(tensor=wvec, offset=q4 * qn, ap=[[0, 128], [1, qn]])
                nc.sync.dma_start(out=w1b[:, q4 * qn:(q4 + 1) * qn], in_=bap)
            whall = consts.tile([128, nch, hid + 1], BF16)
            nc.vector.memset(whall[:, :, hid:hid + 1], 1.0)

            # ---- Wh = x @ W  (column-major loads so chunk 0 is ready fast;
            # x streams on the Act HWDGE queue, masks own the SP queue) ----
            with tc.tile_pool(name="xpool", bufs=1) as xpool:
                xsb = xpool.tile([128, kch, n], BF16)
                for q8 in range(8):
                    sl = slice(q8 * (n // 8), (q8 + 1) * (n // 8))
                    for kc in range(kch):
                        nc.scalar.dma_start(out=xsb[:, kc, sl],
                                            in_=xT[kc * 128:(kc + 1) * 128, sl])
                with tc.tile_pool(name="whps", bufs=4, space="PSUM") as whps:
                    for nch_i in range(nch):
                        pw = whps.tile([128, hid], F32)
                        for kc in range(kch):
                            nc.tensor.matmul(
                                out=pw[:],
                                lhsT=xsb[:, kc, nch_i * 128:(nch_i + 1) * 128],
                                rhs=wsb[:, kc, :],
                                start=(kc == 0), stop=(kc == kch - 1))
                        nc.scalar.activation(out=whall[:, nch_i, 0:hid],
                                             in_=pw[:], func=AF.Copy)

            outTb = consts.tile([hid + 1, n], BF16)

            # ---- attention: z = max(w1b*u2[j], v2[j]) * mask; two j-chunks
            # per iteration share one wide mask-multiply (halves the per-op
            # DVE fixed overhead) and the mask streams on the Act HWDGE
            # queue so it never queues behind the feature loads ----
            with (
                tc.tile_pool(name="mpool", bufs=3) as mpool,
                tc.tile_pool(name="ampool", bufs=2) as ampool,
                tc.tile_pool(name="ppool", bufs=3) as ppool,
                tc.tile_pool(name="atps", bufs=nib, space="PSUM") as atps,
            ):
                pss = [atps.tile([hid + 1, ibw], F32, name=f"pss{_i}", tag="pss")
                       for _i in range(nib)]
                for jp in range(nch // 2):
                    mt = mpool.tile([128, 2, n], BF16)
                    mx = ampool.tile([128, 2, n], BF16)
                    pt = ppool.tile([128, 2, n], BF16)
                    for k in range(2):
                        jc = 2 * jp + k
                        nc.sync.dma_start(out=mt[:, k, :],
                                          in_=maskT[jc * 128:(jc + 1) * 128, :])
                        nc.vector.tensor_scalar(
                            mx[:, k, :], w1b[:], u2c[:, jc:jc + 1],
                            v2c[:, jc:jc + 1], OP.mult, OP.max)
                    nc.vector.tensor_mul(pt[:], mx[:], mt[:])
                    for k in range(2):
                        jc = 2 * jp + k
                        for ib in range(nib):
                            nc.tensor.matmul(
                                out=pss[ib][:],
                                lhsT=whall[:, jc, :],
                                rhs=pt[:, k, ib * ibw:(ib + 1) * ibw],
                                start=(jc == 0), stop=(jc == nch - 1))

                # ---- epilogue: normalize + elu, out h_T [hid, n] bf16 ----
                # psum->sbuf copies spread over three engines; the rowsum
                # block streams to DRAM as each copy lands
                for ib in range(nib):
                    if ib % 2 == 0:
                        nc.scalar.activation(
                            out=outTb[:, ib * ibw:(ib + 1) * ibw],
                            in_=pss[ib][:], func=AF.Copy)
                    else:
                        nc.vector.tensor_copy(
                            out=outTb[:, ib * ibw:(ib + 1) * ibw],
                            in_=pss[ib][:])
                    nc.sync.dma_start(
                        out=bass.AP(tensor=scrR, offset=ib * ibw,
                                    ap=[[ibw, 1], [1, ibw]]),
                        in_=outTb[hid:hid + 1, ib * ibw:(ib + 1) * ibw])
            # reciprocal of rowsums: bounce via DRAM to reshape the [1, n]
            # row onto 128 partitions (cheap DVE recip), then broadcast
            r128 = consts.tile([128, nch], BF16)
            nc.sync.dma_start(out=r128[:],
                              in_=bass.AP(tensor=scrR, offset=0,
                                          ap=[[nch, 128], [1, nch]]))
            rb128 = consts.tile([128, nch], BF16)
            with nc.allow_low_precision(reason="softmax denom reciprocal to bf16"):
                nc.vector.reciprocal(out=rb128[:], in_=r128[:])
            nc.sync.dma_start(out=_flat_write_ap(scrRb, 128, nch), in_=rb128[:])
            rsb = consts.tile([hid, n], BF16)
            for q4 in range(4):
                qn = n // 4
                bap = bass.AP(tensor=scrRb, offset=q4 * qn, ap=[[0, hid], [1, qn]])
                nc.sync.dma_start(out=rsb[:, q4 * qn:(q4 + 1) * qn], in_=bap)
            # elu in two column halves so the vector/scalar chain pipelines
            with tc.tile_pool(name="ep1", bufs=2) as ep1:
                hn = n // 2
                for q2 in range(2):
                    sl = slice(q2 * hn, (q2 + 1) * hn)
                    hv = ep1.tile([hid, hn], BF16, name="hv")
                    nc.vector.tensor_mul(hv[:], outTb[0:hid, sl], rsb[:, sl])
                    rp = ep1.tile([hid, hn], BF16, name="rp")
                    nc.vector.tensor_scalar_max(rp[:], hv[:], 0.0)
                    em = ep1.tile([hid, hn], BF16, name="em")
                    nc.scalar.activation(out=em[:], in_=hv[:], func=AF.Exp)
                    mn = ep1.tile([hid, hn], BF16, name="mn")
                    nc.vector.tensor_scalar(mn[:], em[:], 1.0, -1.0, OP.min, OP.add)
                    hsb = ep1.tile([hid, hn], BF16, name="hsb")
                    nc.vector.tensor_add(hsb[:], mn[:], rp[:])
                    for q4 in range(2):
                        sl2 = slice(q2 * hn + q4 * (hn // 2),
                                    q2 * hn + (q4 + 1) * (hn // 2))
                        nc.sync.dma_start(out=hout[:, sl2],
                                          in_=hsb[:, q4 * (hn // 2):(q4 + 1) * (hn // 2)])

    nc.compile()
    return nc


def build_phase2(n=N, hfull=HEADS * HID, fout=FOUT):
    """Per-core: hT [hfull, n] bf16, m2 [n, n/NCORES] bf16,
    wocat [hfull, fout] bf16, wrow [n/NCORES] bf16, u2i/v2i [128, n/128] f32
    -> out [n/NCORES, fout] f32 (log_softmax rows)."""
    nc = bacc.Bacc("TRN2", target_bir_lowering=False, debug=False,
                   enable_asserts=False)
    rows = n // NCORES        # rows this core owns
    kch = hfull // 128
    nch = n // 128
    rch = rows // 128         # output 128-row chunks
    fb = fout // 128          # 128-col blocks of fout

    hT = nc.dram_tensor("hT", [hfull, n], BF16, kind="ExternalInput")
    m2 = nc.dram_tensor("m2", [n, rows], BF16, kind="ExternalInput")
    wocat = nc.dram_tensor("wocat", [hfull, fout], BF16, kind="ExternalInput")
    wrow = nc.dram_tensor("wrow", [rows], BF16, kind="ExternalInput")
    u2i = nc.dram_tensor("u2i", [128, nch], F32, kind="ExternalInput")
    v2i = nc.dram_tensor("v2i", [128, nch], F32, kind="ExternalInput")
    out = nc.dram_tensor("out", [rows, fout], F32, kind="ExternalOutput")

    with tile.TileContext(nc) as tc:
        with tc.tile_pool(name="consts", bufs=1) as consts:
            id128 = consts.tile([128, 128], F32)
            make_identity(nc, id128[:])
            wosb = consts.tile([128, kch, fout], BF16)
            for kc in range(kch):
                nc.sync.dma_start(out=wosb[:, kc, :],
                                  in_=wocat[kc * 128:(kc + 1) * 128, :])
            u2c = consts.tile([128, nch], F32)
            v2c = consts.tile([128, nch], F32)
            nc.sync.dma_start(out=u2c[:], in_=u2i[:, :])
            nc.sync.dma_start(out=v2c[:], in_=v2i[:, :])
            w2b = consts.tile([128, rows], BF16)
            nc.sync.dma_start(out=w2b[:],
                              in_=bass.AP(tensor=wrow, offset=0,
                                          ap=[[0, 128], [1, rows]]))
            hsb = consts.tile([128, kch, n], BF16)
            for q8 in range(8):
                sl = slice(q8 * (n // 8), (q8 + 1) * (n // 8))
                for kc in range(kch):
                    nc.sync.dma_start(out=hsb[:, kc, sl],
                                      in_=hT[kc * 128:(kc + 1) * 128, sl])

            woall = consts.tile([128, nch, fout + 1], BF16)
            nc.vector.memset(woall[:, :, fout:fout + 1], 1.0)

            # ---- interleaved: Wh2 chunk jc, then attention tile jc ----
            with (
                tc.tile_pool(name="w2ps", bufs=2, space="PSUM") as w2ps,
                tc.tile_pool(name="m2pool", bufs=4) as m2pool,
                tc.tile_pool(name="am2pool", bufs=3) as am2pool,
                tc.tile_pool(name="p2pool", bufs=4) as p2pool,
                tc.tile_pool(name="sfpool", bufs=fb) as sfpool,
                tc.tile_pool(name="a2ps", bufs=fb, space="PSUM") as a2psf,
                tc.tile_pool(name="a2psr", bufs=1, space="PSUM") as a2psr,
            ):
                psf = [a2psf.tile([128, rows], F32, name=f"psf{_i}", tag="psf") for _i in range(fb)]
                psr = a2psr.tile([1, rows], F32)
                pts = {}

                def _attn(jc):
                    pt = pts.pop(jc)
                    for f in range(fb):
                        nc.tensor.matmul(
                            out=psf[f][:],
                            lhsT=woall[:, jc, f * 128:(f + 1) * 128],
                            rhs=pt[:],
                            start=(jc == 0), stop=(jc == nch - 1))
                    nc.tensor.matmul(
                        out=psr[:],
                        lhsT=woall[:, jc, fout:fout + 1],
                        rhs=pt[:],
                        start=(jc == 0), stop=(jc == nch - 1))

                for jc in range(nch):
                    pw = w2ps.tile([128, fout], F32)
                    for kc in range(kch):
                        nc.tensor.matmul(
                            out=pw[:],
                            lhsT=hsb[:, kc, jc * 128:(jc + 1) * 128],
                            rhs=wosb[:, kc, :],
                            start=(kc == 0), stop=(kc == kch - 1))
                    nc.scalar.activation(out=woall[:, jc, 0:fout],
                                         in_=pw[:], func=AF.Copy)

                    mt = m2pool.tile([128, rows], BF16)
                    nc.scalar.dma_start(out=mt[:], in_=m2[jc * 128:(jc + 1) * 128, :])
                    mx = am2pool.tile([128, rows], BF16)
                    nc.vector.tensor_scalar(
                        mx[:], w2b[:], u2c[:, jc:jc + 1], v2c[:, jc:jc + 1],
                        OP.mult, OP.max)
                    pt = p2pool.tile([128, rows], BF16)
                    nc.vector.tensor_mul(pt[:], mx[:], mt[:])
                    pts[jc] = pt
                    # attention for chunk jc-1: its woall scalar-copy overlaps
                    # chunk jc's Wh2 matmuls, so the PE stream never bubbles
                    if jc >= 1:
                        _attn(jc - 1)
                _attn(nch - 1)

                sf = [sfpool.tile([128, rows], F32, name=f"sf{_i}", tag="sf") for _i in range(fb)]
                sr = consts.tile([1, rows], F32)
                for f in range(fb):
                    nc.vector.tensor_copy(out=sf[f][:], in_=psf[f][:])
                nc.vector.tensor_copy(out=sr[:], in_=psr[:])

                # ---- epilogue: per 128-row chunk transpose, norm, elu;
                # log_softmax without max-shift (logits are small) and with
                # Exp/Ln batched to avoid act-table thrash ----
                ones11 = consts.tile([1, 1], F32)
                nc.vector.memset(ones11[:], 1.0)
                helu = consts.tile([128, rch, fout], BF16)
                smv = consts.tile([128, rch], F32)
                with (
                    tc.tile_pool(name="ocps", bufs=2, space="PSUM") as ocps,
                    tc.tile_pool(name="ep", bufs=3) as ep,
                ):
                    for ic in range(rch):
                        oc = ocps.tile([128, fout + 1], F32)
                        for f in range(fb):
                            nc.tensor.transpose(
                                out=oc[:, f * 128:(f + 1) * 128],
                                in_=sf[f][:, ic * 128:(ic + 1) * 128],
                                identity=id128[:])
                        nc.tensor.matmul(
                            out=oc[:, fout:fout + 1],
                            lhsT=sr[0:1, ic * 128:(ic + 1) * 128],
                            rhs=ones11[:], start=True, stop=True)
                        rc = ep.tile([128, 1], F32)
                        nc.vector.reciprocal(out=rc[:], in_=oc[:, fout:fout + 1])
                        an = ep.tile([128, fout], BF16)
                        nc.vector.tensor_scalar_mul(an[:], oc[:, 0:fout], rc[:])
                        rp = ep.tile([128, fout], BF16)
                        nc.vector.tensor_scalar_max(rp[:], an[:], 0.0)
                        emt = ep.tile([128, fout], BF16)
                        nc.scalar.activation(out=emt[:], in_=an[:], func=AF.Exp)
                        mn = ep.tile([128, fout], BF16)
                        nc.vector.tensor_scalar(mn[:], emt[:], 1.0, -1.0,
                                                OP.min, OP.add)
                        nc.vector.tensor_add(helu[:, ic, :], mn[:], rp[:])
                        ex2 = ep.tile([128, fout], BF16)
                        nc.scalar.activation(out=ex2[:], in_=helu[:, ic, :],
                                             func=AF.Exp,
                                             accum_out=smv[:, ic:ic + 1])
                    lnv = consts.tile([128, rch], F32)
                    nc.scalar.activation(out=lnv[:], in_=smv[:], func=AF.Ln)
                    for ic in range(rch):
                        fin = ep.tile([128, fout], F32, name="fin")
                        nc.vector.tensor_scalar_sub(fin[:], helu[:, ic, :],
                                                    lnv[:, ic:ic + 1])
                        nc.sync.dma_start(out=out[ic * 128:(ic + 1) * 128, :], in_=fin[:])

    nc.compile()
    return nc


_CACHE = {}


def _get_programs():
    if "p1" not in _CACHE:
        _CACHE["p1"] = build_phase1()
        _CACHE["p2"] = build_phase2()
    return _CACHE["p1"], _CACHE["p2"]


def _pp_scalars(f2):
    """[N] f32 -> [128, N/128] f32 per-partition layout (p, c) = f[c*128+p]."""
    nch = f2.shape[0] // 128
    return np.ascontiguousarray(f2.reshape(nch, 128).T.astype(np.float32))


def make_in1(x, adj, W_heads, a1_heads, a2_heads):
    bf = ml_dtypes.bfloat16
    xT = np.ascontiguousarray(x.T).astype(bf)
    maskT = np.ascontiguousarray((adj > 0).T.astype(np.float32)).astype(bf)
    in1 = []
    for h in range(NCORES):
        f1 = x @ (W_heads[h] @ a1_heads[h])
        f2 = x @ (W_heads[h] @ a2_heads[h])
        in1.append({
            "xT": xT, "maskT": maskT,
            "wcat": np.ascontiguousarray(W_heads[h].astype(bf)),
            "wvec": np.exp((1.0 - ALPHA) * f1).astype(bf),
            "u2i": _pp_scalars(np.exp(f2)),
            "v2i": _pp_scalars(np.exp(ALPHA * f2)),
        })
    return in1, maskT


def make_in2(hT, maskT, W_out, a1_out, a2_out):
    bf = ml_dtypes.bfloat16
    h32 = hT.astype(np.float32).T           # [N, hfull]
    f1o = h32 @ (W_out @ a1_out)
    f2o = h32 @ (W_out @ a2_out)
    wocat = np.ascontiguousarray(W_out.astype(bf))
    u2o = _pp_scalars(np.exp(f2o))
    v2o = _pp_scalars(np.exp(ALPHA * f2o))
    wo_full = np.exp((1.0 - ALPHA) * f1o).astype(bf)
    rows = N // NCORES
    in2 = []
    for c in range(NCORES):
        in2.append({
            "hT": hT,
            "m2": np.ascontiguousarray(maskT[:, c * rows:(c + 1) * rows]),
            "wocat": wocat,
            "wrow": np.ascontiguousarray(wo_full[c * rows:(c + 1) * rows]),
            "u2i": u2o, "v2i": v2o,
        })
    return in2


def kernel(x, adj, W_heads, a1_heads, a2_heads, W_out, a1_out, a2_out, **_):
    x = np.asarray(x, dtype=np.float32)
    adj = np.asarray(adj)
    W_heads = np.asarray(W_heads, dtype=np.float32)
    a1_heads = np.asarray(a1_heads, dtype=np.float32)
    a2_heads = np.asarray(a2_heads, dtype=np.float32)
    W_out = np.asarray(W_out, dtype=np.float32)
    a1_out = np.asarray(a1_out, dtype=np.float32)
    a2_out = np.asarray(a2_out, dtype=np.float32)

    bf = ml_dtypes.bfloat16
    p1, p2 = _get_programs()

    in1, maskT = make_in1(x, adj, W_heads, a1_heads, a2_heads)
    r1 = run_bass_kernel_spmd(p1, in1, core_ids=list(range(NCORES))).results

    hT = np.concatenate([r1[h]["hout"] for h in range(NCORES)], axis=0)
    hT = np.ascontiguousarray(hT.astype(bf))

    in2 = make_in2(hT, maskT, W_out, a1_out, a2_out)
    r2 = run_bass_kernel_spmd(p2, in2, core_ids=list(range(NCORES))).results

    out = np.concatenate([r2[c]["out"] for c in range(NCORES)], axis=0)
    return out.astype(np.float32)


# revision 14
# speedup vs baseline: 1.1612x; 1.1612x over previous
"""GAT (2-layer, 8-head) Trainium2 kernel, 8-core SPMD.

Phase 1: head-parallel — core h computes head h's GAT layer over the full
  graph.  Uses the identity  exp(lrelu(f1_i + f2_j)) = v1_i * max(w_i*u2_j, v2_j)
  with w = exp((1-a)f1), u2 = exp(f2), v2 = exp(a*f2); the v1_i factor is a
  per-row scale that cancels in the softmax normalization, so the [N,N]
  unnormalized attention needs ONE 4x-mode tensor_scalar (mult+max against
  two per-partition scalars) and ONE 2x-mode tensor_tensor mask multiply per
  128-row tile (mask multiply split between DVE and GpSimd).  The tiny f1/f2
  vectors (x @ (W@a), O(N*F)) are folded on the host like the W@a folds, so
  the attention stream starts immediately.  A zero-weight dummy matmul gates
  each 4-tile block of PE work so the tensor engine runs in long bursts and
  ramps out of its low-power state.
Phase 2: row-parallel — host gathers h_T [512, N] (bf16), every core computes
  the full Wh2 = h@W_out chunk-by-chunk interleaved with its attention
  matmuls (keeps PE continuously busy), then elu + log_softmax (no max-shift;
  logits are small) for its own N/8-row slice.
"""

import sys

for p in ("/opt/trn_rl_repo", "/opt/pypackages"):
    if p not in sys.path:
        sys.path.append(p)

import numpy as np
import ml_dtypes

import concourse.bass as bass
import concourse.bacc as bacc
import concourse.tile as tile
from concourse import mybir
from concourse.bass_utils import run_bass_kernel_spmd
from concourse.masks import make_identity

BF16 = mybir.dt.bfloat16
F32 = mybir.dt.float32
AX = mybir.AxisListType
OP = mybir.AluOpType
AF = mybir.ActivationFunctionType

N, FIN, HID, HEADS, FOUT = 4096, 512, 64, 8, 256
NCORES = 8
ALPHA = 0.2


def _flat_write_ap(t, rows, cols):
    return bass.AP(tensor=t, offset=0, ap=[[cols, rows], [1, cols]])


def build_phase1(n=N, fin=FIN, hid=HID):
    """Per-core: xT [fin, n] bf16, maskT [n, n] bf16, wcat [fin, hid] bf16,
    wvec [n] bf16 (= exp((1-a)f1)), u2i/v2i [128, n/128] f32
    -> hout [hid, n] bf16 (elu'd, normalized head features, transposed)."""
    nc = bacc.Bacc("TRN2", target_bir_lowering=False, debug=False,
                   enable_asserts=False)
    kch = fin // 128          # contraction chunks for x@W
    nch = n // 128            # 128-row chunks of nodes
    nib = max(1, n // 512)    # 512-col i-blocks for PSUM banks
    ibw = min(n, 512)

    xT = nc.dram_tensor("xT", [fin, n], BF16, kind="ExternalInput")
    maskT = nc.dram_tensor("maskT", [n, n], BF16, kind="ExternalInput")
    wcat = nc.dram_tensor("wcat", [fin, hid], BF16, kind="ExternalInput")
    wvec = nc.dram_tensor("wvec", [n], BF16, kind="ExternalInput")
    u2i = nc.dram_tensor("u2i", [128, nch], F32, kind="ExternalInput")
    v2i = nc.dram_tensor("v2i", [128, nch], F32, kind="ExternalInput")
    hout = nc.dram_tensor("hout", [hid, n], BF16, kind="ExternalOutput")
    scrR = nc.dram_tensor("scrR", [n], BF16)
    scrRb = nc.dram_tensor("scrRb", [n], BF16)

    with tile.TileContext(nc) as tc:
        with tc.tile_pool(name="consts", bufs=1) as consts:
            wsb = consts.tile([128, kch, hid], BF16)
            for kc in range(kch):
                nc.sync.dma_start(out=wsb[:, kc, :], in_=wcat[kc * 128:(kc + 1) * 128, :])
            u2c = consts.tile([128, nch], F32)
            v2c = consts.tile([128, nch], F32)
            nc.sync.dma_start(out=u2c[:], in_=u2i[:, :])
            nc.sync.dma_start(out=v2c[:], in_=v2i[:, :])
            w1b = consts.tile([128, n], BF16)
            for q4 in range(4):
                qn = n // 4
                bap = bass.AP(tensor=wvec, offset=q4 * qn, ap=[[0, 128], [1, qn]])
                nc.sync.dma_start(out=w1b[:, q4 * qn:(q4 + 1) * qn], in_=bap)
            whall = consts.tile([128, nch, hid + 1], BF16)
            nc.vector.memset(whall[:, :, hid:hid + 1], 1.0)
            outTb = consts.tile([hid + 1, n], BF16)

            npair = nch // 2
            with (
                tc.tile_pool(name="mpool", bufs=5) as mpool,
                tc.tile_pool(name="ptpool", bufs=3) as ptpool,
            ):
                # The scalar engine's stream carries only mask-DMA issues and
                # the whall psum->sbuf copies, interleaved so the mask stream
                # (on the Act HWDGE queue) never starves the DVE while the
                # vector stream stays pure attention math.  The attention
                # pools sit below xsb in SBUF, so mask tiles land while the
                # Wh matmuls still read x.
                mts = []

                def issue_pair(jp):
                    mt = mpool.tile([128, 2, n], BF16, name="mt")
                    for k in range(2):
                        jc = 2 * jp + k
                        nc.scalar.dma_start(out=mt[:, k, :],
                                            in_=maskT[jc * 128:(jc + 1) * 128, :])
                    mts.append(mt)

                for jp in range(4):
                    issue_pair(jp)

                # ---- Wh = x @ W  (column-major loads so chunk 0 is ready
                # fast; x and everything else stream on the SP queue) ----
                with tc.tile_pool(name="xpool", bufs=1) as xpool:
                    xsb = xpool.tile([128, kch, n], BF16)
                    for q8 in range(8):
                        sl = slice(q8 * (n // 8), (q8 + 1) * (n // 8))
                        for kc in range(kch):
                            nc.sync.dma_start(out=xsb[:, kc, sl],
                                              in_=xT[kc * 128:(kc + 1) * 128, sl])
                    with tc.tile_pool(name="whps", bufs=4, space="PSUM") as whps:
                        for nch_i in range(nch):
                            pw = whps.tile([128, hid], F32)
                            for kc in range(kch):
                                nc.tensor.matmul(
                                    out=pw[:],
                                    lhsT=xsb[:, kc, nch_i * 128:(nch_i + 1) * 128],
                                    rhs=wsb[:, kc, :],
                                    start=(kc == 0), stop=(kc == kch - 1))
                            nc.scalar.activation(out=whall[:, nch_i, 0:hid],
                                                 in_=pw[:], func=AF.Copy)
                            if nch_i % 2 == 0 and 4 + nch_i // 2 <= 9:
                                issue_pair(4 + nch_i // 2)
                for jp in range(10, npair):
                    issue_pair(jp)

                # ---- attention: z = max(w1b*u2[j], v2[j]) * mask; two
                # j-chunks per iteration share one wide in-place mask
                # multiply ----
                with tc.tile_pool(name="atps", bufs=nib, space="PSUM") as atps:
                    pss = [atps.tile([hid + 1, ibw], F32, name=f"pss{_i}", tag="pss")
                           for _i in range(nib)]
                    for jp in range(npair):
                        pt = ptpool.tile([128, 2, n], BF16)
                        for k in range(2):
                            jc = 2 * jp + k
                            nc.vector.tensor_scalar(
                                pt[:, k, :], w1b[:], u2c[:, jc:jc + 1],
                                v2c[:, jc:jc + 1], OP.mult, OP.max)
                        nc.vector.tensor_mul(pt[:], pt[:], mts[jp][:])
                        for k in range(2):
                            jc = 2 * jp + k
                            for ib in range(nib):
                                nc.tensor.matmul(
                                    out=pss[ib][:],
                                    lhsT=whall[:, jc, :],
                                    rhs=pt[:, k, ib * ibw:(ib + 1) * ibw],
                                    start=(jc == 0), stop=(jc == nch - 1))

                    # ---- epilogue: normalize + elu, out h_T [hid, n] ----
                    # psum->sbuf copies split over two engines; the rowsum
                    # block streams to DRAM as each copy lands
                    for ib in range(nib):
                        if ib % 2 == 0:
                            nc.scalar.activation(
                                out=outTb[:, ib * ibw:(ib + 1) * ibw],
                                in_=pss[ib][:], func=AF.Copy)
                        else:
                            nc.vector.tensor_copy(
                                out=outTb[:, ib * ibw:(ib + 1) * ibw],
                                in_=pss[ib][:])
                        nc.sync.dma_start(
                            out=bass.AP(tensor=scrR, offset=ib * ibw,
                                        ap=[[ibw, 1], [1, ibw]]),
                            in_=outTb[hid:hid + 1, ib * ibw:(ib + 1) * ibw])
            # reciprocal of rowsums: bounce via DRAM to reshape the [1, n]
            # row onto 128 partitions (cheap DVE recip), then broadcast
            r128 = consts.tile([128, nch], BF16)
            nc.sync.dma_start(out=r128[:],
                              in_=bass.AP(tensor=scrR, offset=0,
                                          ap=[[nch, 128], [1, nch]]))
            rb128 = consts.tile([128, nch], BF16)
            with nc.allow_low_precision(reason="softmax denom reciprocal to bf16"):
                nc.vector.reciprocal(out=rb128[:], in_=r128[:])
            nc.sync.dma_start(out=_flat_write_ap(scrRb, 128, nch), in_=rb128[:])
            rsb = consts.tile([hid, n], BF16)
            for q4 in range(4):
                qn = n // 4
                bap = bass.AP(tensor=scrRb, offset=q4 * qn, ap=[[0, hid], [1, qn]])
                nc.sync.dma_start(out=rsb[:, q4 * qn:(q4 + 1) * qn], in_=bap)
            # elu in two column halves so the vector/scalar chain pipelines
            with tc.tile_pool(name="ep1", bufs=2) as ep1:
                hn = n // 2
                for q2 in range(2):
                    sl = slice(q2 * hn, (q2 + 1) * hn)
                    hv = ep1.tile([hid, hn], BF16, name="hv")
                    nc.vector.tensor_mul(hv[:], outTb[0:hid, sl], rsb[:, sl])
                    rp = ep1.tile([hid, hn], BF16, name="rp")
                    nc.vector.tensor_scalar_max(rp[:], hv[:], 0.0)
                    em = ep1.tile([hid, hn], BF16, name="em")
                    nc.scalar.activation(out=em[:], in_=hv[:], func=AF.Exp)
                    mn = ep1.tile([hid, hn], BF16, name="mn")
                    nc.vector.tensor_scalar(mn[:], em[:], 1.0, -1.0, OP.min, OP.add)
                    hsb = ep1.tile([hid, hn], BF16, name="hsb")
                    nc.vector.tensor_add(hsb[:], mn[:], rp[:])
                    for q4 in range(2):
                        sl2 = slice(q2 * hn + q4 * (hn // 2),
                                    q2 * hn + (q4 + 1) * (hn // 2))
                        nc.sync.dma_start(out=hout[:, sl2],
                                          in_=hsb[:, q4 * (hn // 2):(q4 + 1) * (hn // 2)])

    nc.compile()
    return nc


def build_phase2(n=N, hfull=HEADS * HID, fout=FOUT):
    """Per-core: hT [hfull, n] bf16, m2 [n, n/NCORES] bf16,
    wocat [hfull, fout] bf16, wrow [n/NCORES] bf16, u2i/v2i [128, n/128] f32
    -> out [n/NCORES, fout] f32 (log_softmax rows)."""
    nc = bacc.Bacc("TRN2", target_bir_lowering=False, debug=False,
                   enable_asserts=False)
    rows = n // NCORES        # rows this core owns
    kch = hfull // 128
    nch = n // 128
    rch = rows // 128         # output 128-row chunks
    fb = fout // 128          # 128-col blocks of fout

    hT = nc.dram_tensor("hT", [hfull, n], BF16, kind="ExternalInput")
    m2 = nc.dram_tensor("m2", [n, rows], BF16, kind="ExternalInput")
    wocat = nc.dram_tensor("wocat", [hfull, fout], BF16, kind="ExternalInput")
    wrow = nc.dram_tensor("wrow", [rows], BF16, kind="ExternalInput")
    u2i = nc.dram_tensor("u2i", [128, nch], F32, kind="ExternalInput")
    v2i = nc.dram_tensor("v2i", [128, nch], F32, kind="ExternalInput")
    out = nc.dram_tensor("out", [rows, fout], F32, kind="ExternalOutput")

    with tile.TileContext(nc) as tc:
        with tc.tile_pool(name="consts", bufs=1) as consts:
            id128 = consts.tile([128, 128], F32)
            make_identity(nc, id128[:])
            wosb = consts.tile([128, kch, fout], BF16)
            for kc in range(kch):
                nc.sync.dma_start(out=wosb[:, kc, :],
                                  in_=wocat[kc * 128:(kc + 1) * 128, :])
            u2c = consts.tile([128, nch], F32)
            v2c = consts.tile([128, nch], F32)
            nc.sync.dma_start(out=u2c[:], in_=u2i[:, :])
            nc.sync.dma_start(out=v2c[:], in_=v2i[:, :])
            w2b = consts.tile([128, rows], BF16)
            nc.sync.dma_start(out=w2b[:],
                              in_=bass.AP(tensor=wrow, offset=0,
                                          ap=[[0, 128], [1, rows]]))
            hsb = consts.tile([128, kch, n], BF16)
            for q8 in range(8):
                sl = slice(q8 * (n // 8), (q8 + 1) * (n // 8))
                for kc in range(kch):
                    nc.sync.dma_start(out=hsb[:, kc, sl],
                                      in_=hT[kc * 128:(kc + 1) * 128, sl])

            woall = consts.tile([128, nch, fout + 1], BF16)
            nc.vector.memset(woall[:, :, fout:fout + 1], 1.0)

            # ---- interleaved: Wh2 chunk jc, then attention tile jc ----
            with (
                tc.tile_pool(name="w2ps", bufs=2, space="PSUM") as w2ps,
                tc.tile_pool(name="m2pool", bufs=4) as m2pool,
                tc.tile_pool(name="am2pool", bufs=3) as am2pool,
                tc.tile_pool(name="p2pool", bufs=4) as p2pool,
                tc.tile_pool(name="sfpool", bufs=fb) as sfpool,
                tc.tile_pool(name="a2ps", bufs=fb, space="PSUM") as a2psf,
                tc.tile_pool(name="a2psr", bufs=1, space="PSUM") as a2psr,
            ):
                psf = [a2psf.tile([128, rows], F32, name=f"psf{_i}", tag="psf") for _i in range(fb)]
                psr = a2psr.tile([1, rows], F32)
                pts = {}

                def _attn(jc):
                    pt = pts.pop(jc)
                    for f in range(fb):
                        nc.tensor.matmul(
                            out=psf[f][:],
                            lhsT=woall[:, jc, f * 128:(f + 1) * 128],
                            rhs=pt[:],
                            start=(jc == 0), stop=(jc == nch - 1))
                    nc.tensor.matmul(
                        out=psr[:],
                        lhsT=woall[:, jc, fout:fout + 1],
                        rhs=pt[:],
                        start=(jc == 0), stop=(jc == nch - 1))

                for jc in range(nch):
                    pw = w2ps.tile([128, fout], F32)
                    for kc in range(kch):
                        nc.tensor.matmul(
                            out=pw[:],
                            lhsT=hsb[:, kc, jc * 128:(jc + 1) * 128],
                            rhs=wosb[:, kc, :],
                            start=(kc == 0), stop=(kc == kch - 1))
                    nc.scalar.activation(out=woall[:, jc, 0:fout],
                                         in_=pw[:], func=AF.Copy)

                    mt = m2pool.tile([128, rows], BF16)
                    nc.scalar.dma_start(out=mt[:], in_=m2[jc * 128:(jc + 1) * 128, :])
                    mx = am2pool.tile([128, rows], BF16)
                    nc.vector.tensor_scalar(
                        mx[:], w2b[:], u2c[:, jc:jc + 1], v2c[:, jc:jc + 1],
                        OP.mult, OP.max)
                    pt = p2pool.tile([128, rows], BF16)
                    nc.vector.tensor_mul(pt[:], mx[:], mt[:])
                    pts[jc] = pt
                    # attention for chunk jc-1: its woall scalar-copy overlaps
                    # chunk jc's Wh2 matmuls, so the PE stream never bubbles
                    if jc >= 1:
                        _attn(jc - 1)
                _attn(nch - 1)

                sf = [sfpool.tile([128, rows], F32, name=f"sf{_i}", tag="sf") for _i in range(fb)]
                sr = consts.tile([1, rows], F32)
                for f in range(fb):
                    nc.vector.tensor_copy(out=sf[f][:], in_=psf[f][:])
                nc.vector.tensor_copy(out=sr[:], in_=psr[:])

                # ---- epilogue: per 128-row chunk transpose, norm, elu;
                # log_softmax without max-shift (logits are small) and with
                # Exp/Ln batched to avoid act-table thrash ----
                ones11 = consts.tile([1, 1], F32)
                nc.vector.memset(ones11[:], 1.0)
                helu = consts.tile([128, rch, fout], BF16)
                smv = consts.tile([128, rch], F32)
                with (
                    tc.tile_pool(name="ocps", bufs=2, space="PSUM") as ocps,
                    tc.tile_pool(name="ep", bufs=3) as ep,
                ):
                    for ic in range(rch):
                        oc = ocps.tile([128, fout + 1], F32)
                        for f in range(fb):
                            nc.tensor.transpose(
                                out=oc[:, f * 128:(f + 1) * 128],
                                in_=sf[f][:, ic * 128:(ic + 1) * 128],
                                identity=id128[:])
                        nc.tensor.matmul(
                            out=oc[:, fout:fout + 1],
                            lhsT=sr[0:1, ic * 128:(ic + 1) * 128],
                            rhs=ones11[:], start=True, stop=True)
                        rc = ep.tile([128, 1], F32)
                        nc.vector.reciprocal(out=rc[:], in_=oc[:, fout:fout + 1])
                        an = ep.tile([128, fout], BF16)
                        nc.vector.tensor_scalar_mul(an[:], oc[:, 0:fout], rc[:])
                        rp = ep.tile([128, fout], BF16)
                        nc.vector.tensor_scalar_max(rp[:], an[:], 0.0)
                        emt = ep.tile([128, fout], BF16)
                        nc.scalar.activation(out=emt[:], in_=an[:], func=AF.Exp)
                        mn = ep.tile([128, fout], BF16)
                        nc.vector.tensor_scalar(mn[:], emt[:], 1.0, -1.0,
                                                OP.min, OP.add)
                        nc.vector.tensor_add(helu[:, ic, :], mn[:], rp[:])
                        ex2 = ep.tile([128, fout], BF16)
                        nc.scalar.activation(out=ex2[:], in_=helu[:, ic, :],
                                             func=AF.Exp,
                                             accum_out=smv[:, ic:ic + 1])
                    lnv = consts.tile([128, rch], F32)
                    nc.scalar.activation(out=lnv[:], in_=smv[:], func=AF.Ln)
                    for ic in range(rch):
                        fin = ep.tile([128, fout], F32, name="fin")
                        nc.vector.tensor_scalar_sub(fin[:], helu[:, ic, :],
                                                    lnv[:, ic:ic + 1])
                        nc.sync.dma_start(out=out[ic * 128:(ic + 1) * 128, :], in_=fin[:])

    nc.compile()
    return nc


_CACHE = {}


def _get_programs():
    if "p1" not in _CACHE:
        _CACHE["p1"] = build_phase1()
        _CACHE["p2"] = build_phase2()
    return _CACHE["p1"], _CACHE["p2"]


def _pp_scalars(f2):
    """[N] f32 -> [128, N/128] f32 per-partition layout (p, c) = f[c*128+p]."""
    nch = f2.shape[0] // 128
    return np.ascontiguousarray(f2.reshape(nch, 128).T.astype(np.float32))


def make_in1(x, adj, W_heads, a1_heads, a2_heads):
    bf = ml_dtypes.bfloat16
    xT = np.ascontiguousarray(x.T).astype(bf)
    maskT = np.ascontiguousarray((adj > 0).T.astype(np.float32)).astype(bf)
    in1 = []
    for h in range(NCORES):
        f1 = x @ (W_heads[h] @ a1_heads[h])
        f2 = x @ (W_heads[h] @ a2_heads[h])
        in1.append({
            "xT": xT, "maskT": maskT,
            "wcat": np.ascontiguousarray(W_heads[h].astype(bf)),
            "wvec": np.exp((1.0 - ALPHA) * f1).astype(bf),
            "u2i": _pp_scalars(np.exp(f2)),
            "v2i": _pp_scalars(np.exp(ALPHA * f2)),
        })
    return in1, maskT


def make_in2(hT, maskT, W_out, a1_out, a2_out):
    bf = ml_dtypes.bfloat16
    h32 = hT.astype(np.float32).T           # [N, hfull]
    f1o = h32 @ (W_out @ a1_out)
    f2o = h32 @ (W_out @ a2_out)
    wocat = np.ascontiguousarray(W_out.astype(bf))
    u2o = _pp_scalars(np.exp(f2o))
    v2o = _pp_scalars(np.exp(ALPHA * f2o))
    wo_full = np.exp((1.0 - ALPHA) * f1o).astype(bf)
    rows = N // NCORES
    in2 = []
    for c in range(NCORES):
        in2.append({
            "hT": hT,
            "m2": np.ascontiguousarray(maskT[:, c * rows:(c + 1) * rows]),
            "wocat": wocat,
            "wrow": np.ascontiguousarray(wo_full[c * rows:(c + 1) * rows]),
            "u2i": u2o, "v2i": v2o,
        })
    return in2


def kernel(x, adj, W_heads, a1_heads, a2_heads, W_out, a1_out, a2_out, **_):
    x = np.asarray(x, dtype=np.float32)
    adj = np.asarray(adj)
    W_heads = np.asarray(W_heads, dtype=np.float32)
    a1_heads = np.asarray(a1_heads, dtype=np.float32)
    a2_heads = np.asarray(a2_heads, dtype=np.float32)
    W_out = np.asarray(W_out, dtype=np.float32)
    a1_out = np.asarray(a1_out, dtype=np.float32)
    a2_out = np.asarray(a2_out, dtype=np.float32)

    bf = ml_dtypes.bfloat16
    p1, p2 = _get_programs()

    in1, maskT = make_in1(x, adj, W_heads, a1_heads, a2_heads)
    r1 = run_bass_kernel_spmd(p1, in1, core_ids=list(range(NCORES))).results

    hT = np.concatenate([r1[h]["hout"] for h in range(NCORES)], axis=0)
    hT = np.ascontiguousarray(hT.astype(bf))

    in2 = make_in2(hT, maskT, W_out, a1_out, a2_out)
    r2 = run_bass_kernel_spmd(p2, in2, core_ids=list(range(NCORES))).results

    out = np.concatenate([r2[c]["out"] for c in range(NCORES)], axis=0)
    return out.astype(np.float32)
